# revision 92
# baseline (speedup 1.0000x reference)
"""DCN (cross+deep) Trainium2 Bass kernel, 8 NeuronCores.

Sharding: data-parallel over batch (2048 rows/core); embedding gather on
host (table never touches the device); cross/deep weights replicated.

Key structure (vs the naive formulation):
  * Cross branch is algebraically collapsed: with a_i = x0 . w_i and
    a_3 = x0 . ow_cross, the full cross stack + its output contribution
    reduce to per-row scalar recurrences:
       S0 = a0; u1 = 1+S0; S1 = u1*a1 + c1; u2 = u1+S1; S2 = u2*a2 + c2;
       T = u2+S2; out_cross = T*a3 + const.
    So the PE computes ONE 7-matmul group ([128,4] lhsT) instead of
    3x7 broadcast matvecs + 7 output matvecs.
  * Deep branch runs in fp8(e4m3) with DoubleRow perf mode: each matmul
    contracts two 128-row k-tiles at 0.5 cycles/output-row. Activations
    are scaled x256 and weights x16 (exact power-of-2 descale in the
    relus), keeping everything in e4m3's normal range.
  * x ships pre-transposed from host in bf16 (cross) + fp8 (deep)
    layouts. Engine split per chunk: ACT = L0 relus + a-copy; DVE =
    L1/L2 relus + final add; Pool = cross recurrence ([4,128] layout,
    brought to partitions 0-3 by a tiny SBUF->SBUF DMA shuffle).
  * L1/L2 run k-pair-outer so they can start as soon as the first two
    producer tiles are relu'd; out_d accumulates in [4,128] PSUM groups
    so the tail is one small DVE add + DMA.
"""

import numpy as np
import ml_dtypes
from contextlib import ExitStack

import concourse.tile as tile
import concourse.mybir as mybir
from concourse import bacc
from concourse.bass_utils import run_bass_kernel_spmd

# ---- problem constants (hardcoded; kernel.py must be self-contained) ----
B, F, E = 16384, 26, 32
NF = 1_000_000
D = F * E                     # 832
DEEP = (1024, 512, 256)
N_CORES = 8
S = B // N_CORES              # 2048 rows per core
CHUNK = 512
NCHUNK = S // CHUNK           # 4
KB = 7                        # bf16 k-tiles (896 = 28 features)
K8 = 8                        # fp8 k-tiles (1024 = 32 features)
FPB, FP8 = 28, 32             # padded feature counts
M0, M1, M2 = DEEP[0] // 128, DEEP[1] // 128, DEEP[2] // 128  # 8, 4, 2
XS, WS = 256.0, 16.0          # fp8 scales for activations / weights

_bf = mybir.dt.bfloat16
_f8 = mybir.dt.float8e4
_f32 = mybir.dt.float32
_np_bf = ml_dtypes.bfloat16
_np_f8 = ml_dtypes.float8_e4m3

_CACHE = {}
DR = mybir.MatmulPerfMode.DoubleRow


# scheduling knobs (swept against the cost-model timeline sim)
# *split: 0 = never, 1 = all chunks, 3 = last chunk only
CFG = dict(l2_dve="alt", y0split=0, l1split=0, l2split=0, dpsb=5, warm_n=7)


def _build_nc(zb=True, zc=True, zo=True):
    """zb: deep biases all zero; zc: cross biases zero; zo: out bias zero."""
    AF = mybir.ActivationFunctionType
    OP = mybir.AluOpType
    nc = bacc.Bacc(
        "TRN2", target_bir_lowering=False, debug=False, num_devices=N_CORES
    )

    # x pre-transposed on host: xtb[p, k*S+b] = bf16(x[b, k*128+p])
    xtb_d = nc.dram_tensor("xtb", [128, KB * S], _bf, kind="ExternalInput")
    # x8[p, k*S+b] = fp8(x[b, k*128+p] * 256)
    xt8_d = nc.dram_tensor("xt8", [128, K8 * S], _f8, kind="ExternalInput")
    # deep weights fp8 (x16): w[p, k, m] = fp8(W[k*128+p, m] * 16)
    w08_d = nc.dram_tensor("w08", [128, K8 * DEEP[0]], _f8, kind="ExternalInput")
    w18_d = nc.dram_tensor("w18", [128, K8 * DEEP[1]], _f8, kind="ExternalInput")
    w28_d = nc.dram_tensor("w28", [128, M1 * DEEP[2]], _f8, kind="ExternalInput")

    # merged small weights bf16: [cwo (28) | owd (2) | obb (1) | vb (1)]
    # vb: rows 0-2 = 1.0, row 3 = 0 -- the asb-copy's per-partition bias,
    # so asb rows become [v0,v1,v2,a3] with v_i = 1 + a_i and the cross
    # recurrence factorizes to out_cross = ((v0*v1 + c1)*v2 + c2) * a3.
    SMW = KB * 4 + M2 + 1 + 1
    smw_d = nc.dram_tensor("smw", [128, SMW], _bf, kind="ExternalInput")
    if not zc:
        sc_d = nc.dram_tensor("sc", [1, 2], _f32, kind="ExternalInput")
    if not zb:
        cst_d = nc.dram_tensor("cst", [128, M0 + M1 + M2], _f32, kind="ExternalInput")
    out_d = nc.dram_tensor("out", [NCHUNK, CHUNK], _f32, kind="ExternalOutput")

    xtb_r = xtb_d[:, :].rearrange("p (k s) -> p k s", k=KB)
    xt8_r = xt8_d[:, :].rearrange("p (k s) -> p k s", k=K8)
    w08_r = w08_d[:, :].rearrange("p (k m) -> p k m", k=K8)
    w18_r = w18_d[:, :].rearrange("p (k m) -> p k m", k=K8)
    w28_r = w28_d[:, :].rearrange("p (k m) -> p k m", k=M1)

    with ExitStack() as ctx:
        tc = ctx.enter_context(tile.TileContext(nc))
        wp = ctx.enter_context(tc.tile_pool(name="wp", bufs=1))
        xbp = ctx.enter_context(tc.tile_pool(name="xbp", bufs=2))
        x8p = ctx.enter_context(tc.tile_pool(name="x8p", bufs=2))
        yp = ctx.enter_context(tc.tile_pool(name="yp", bufs=2))
        asp = ctx.enter_context(tc.tile_pool(name="asp", bufs=2))
        rp = ctx.enter_context(tc.tile_pool(name="rp", bufs=2))
        otp = ctx.enter_context(tc.tile_pool(name="otp", bufs=2))
        dps = ctx.enter_context(
            tc.tile_pool(name="dps", bufs=CFG["dpsb"], space="PSUM")
        )
        aps = ctx.enter_context(tc.tile_pool(name="aps", bufs=1, space="PSUM"))
        ops = ctx.enter_context(tc.tile_pool(name="ops", bufs=1, space="PSUM"))

        # ---- startup DMA order: smw (tiny, touched by PE preamble), then
        # x chunk 0 + w0 (the L0 critical path).  w0 lives in TWO tiles so
        # L0 m0-3 don't wait on the second DMA (tile dependency tracking is
        # tile-granular). ----
        smw_sb = wp.tile([128, SMW], _bf)
        nc.sync.dma_start(smw_sb[:], smw_d[:, :])
        w08a_sb = wp.tile([128, K8, DEEP[0] // 2], _f8)
        w08b_sb = wp.tile([128, K8, DEEP[0] // 2], _f8)
        w18_sb = wp.tile([128, K8, DEEP[1]], _f8)
        w28_sb = wp.tile([128, M1, DEEP[2]], _f8)
        nc.sync.dma_start(w08a_sb[:], w08_r[:, :, 0:512])
        xt8_0 = x8p.tile([128, K8, CHUNK], _f8, tag="xt8", name="xt8_0")
        nc.sync.dma_start(xt8_0[:], xt8_r[:, :, 0:CHUNK])
        nc.sync.dma_start(w08b_sb[:], w08_r[:, :, 512:1024])

        def w0l(m):  # [128, 2, 128] lhsT slice provider for L0 tile (j pair)
            t = w08a_sb if m < 4 else w08b_sb
            mm = m % 4
            return lambda j: t[:, 2 * j:2 * j + 2, mm * 128:(mm + 1) * 128]

        def cwo(k):  # [128, 4] lhsT for a-pass k-tile
            return smw_sb[:, k * 4:(k + 1) * 4]

        def owd(m):  # [128, 1] deep-out column
            return smw_sb[:, KB * 4 + m:KB * 4 + m + 1]

        obb = smw_sb[:, KB * 4 + M2:KB * 4 + M2 + 1]
        vb = smw_sb[0:4, KB * 4 + M2 + 1:KB * 4 + M2 + 2]
        if not zc:
            sc_sb = wp.tile([1, 2], _f32)
            nc.sync.dma_start(sc_sb[:], sc_d[:, :])
        if not zb:
            cst_sb = wp.tile([128, M0 + M1 + M2], _f32)
            nc.sync.dma_start(cst_sb[:], cst_d[:, :])
            b0_sb = cst_sb[:, 0:M0]
            b1_sb = cst_sb[:, M0:M0 + M1]
            b2_sb = cst_sb[:, M0 + M1:M0 + M1 + M2]

        # ---- preamble: observe ops + PE warm-up (p-state ramp) ----
        obs = wp.tile([128, 8], _f32)
        nc.vector.tensor_copy(obs[:, 0:1], smw_sb[:, 0:1])
        nc.gpsimd.tensor_copy(obs[:, 1:2], smw_sb[:, 0:1])
        if not zc:
            nc.vector.tensor_copy(obs[:, 2:3], sc_sb[0:1, 0:1])
        nc.scalar.activation(obs[:, 3:4], smw_sb[:, 0:1], AF.Copy)
        if not zb:
            nc.scalar.activation(obs[:, 4:5], b0_sb[:, 0:1], AF.Copy)
        warm = wp.tile([128, 512], _bf)
        nc.vector.memset(warm[:], 0.0)
        if not zo:
            ones_sb = wp.tile([128, CHUNK], _bf)
            nc.gpsimd.memset(ones_sb[:], 1.0)
        warm_ps = dps.tile([128, 512], _f32, tag="dps", name="warm_ps")
        for _ in range(CFG["warm_n"]):
            nc.tensor.matmul(
                warm_ps[:], lhsT=warm[:, 0:128], rhs=warm[:], start=True, stop=True
            )
        # NOTE: only touch tensors whose DMAs are emitted BEFORE this point —
        # touching late-loaded weights stalls the in-order PE stream on their
        # DMA semaphores.
        dummy_ps = ops.tile([1, 8], _f32, tag="dummy", bufs=1)
        touch = [
            w08a_sb[:, 0:1, 0:1],
            smw_sb[:, 0:1],
        ]
        if not zo:
            touch.append(ones_sb[:, 0:1])
        for w_ap in touch:
            nc.tensor.matmul(dummy_ps[0:1, 0:1], lhsT=w_ap, rhs=w_ap, start=True, stop=True)

        HH = CHUNK // 2

        def relu(out_ap, ps, scale, bias_col, on_dve, split=False):
            # relu of one [128, CHUNK] psum tile; tiles alternate between ACT
            # and DVE so neither engine queues (inter-arrival 2x427ns > op).
            # split=True: column-halves on both engines -> ~2x lower latency,
            # for tiles that gate the next layer's last matmuls.
            if zb and split:
                nc.scalar.activation(
                    out_ap[:, 0:HH], ps[:, 0:HH], AF.Relu, scale=scale
                )
                nc.vector.tensor_scalar(
                    out_ap[:, HH:], ps[:, HH:], scale, 0.0, OP.mult, OP.max
                )
            elif zb and on_dve:
                nc.vector.tensor_scalar(
                    out_ap[:, :], ps[:, :], scale, 0.0, OP.mult, OP.max
                )
            else:
                nc.scalar.activation(
                    out_ap[:, :], ps[:, :], AF.Relu,
                    bias=0.0 if bias_col is None else bias_col, scale=scale,
                )

        def emit_apass(cc, xtb_t):
            # cross a-pass (bf16): psA rows = [a0, a1, a2, a3]; the +1
            # for v_i = 1 + a_i rides the asb copy as a partition bias
            psA = aps.tile([4, CHUNK], _f32, tag="a", name=f"psA_{cc}")
            for k in range(KB):
                nc.tensor.matmul(
                    psA[:],
                    lhsT=cwo(k),
                    rhs=xtb_t[:, k, :],
                    start=(k == 0),
                    stop=(k == KB - 1),
                )
            asb = asp.tile([4, CHUNK], _bf, tag="asb", name=f"asb_{cc}")
            nc.scalar.activation(asb[:], psA[:], AF.Identity, bias=vb)
            # shuffle all four rows onto partition 0 (engines can't cross
            # partitions; the DMA crossbar can): as1[0,i,b] = a_i[b]
            as1 = asp.tile([1, 4, CHUNK], _bf, tag="as1", name=f"as1_{cc}")
            nc.sync.dma_start(out=as1[:, :, :], in_=asb[:, :])
            return as1

        def emit_cross(cc, as1, eng):
            # cross combine: oc = ((v0*v1 + c1)*v2 + c2) * a3.  DVE at chunk
            # end for pipelined chunks; Pool (idle) for the last chunk so its
            # relus aren't queued behind the chain.
            v0 = as1[:, 0, :]
            v1 = as1[:, 1, :]
            v2 = as1[:, 2, :]
            a3 = as1[:, 3, :]
            p1 = rp.tile([1, CHUNK], _bf, tag="p1", name=f"p1_{cc}")
            eng.tensor_tensor(out=p1[:], in0=v0, in1=v1, op=OP.mult)
            if not zc:
                eng.tensor_scalar_add(p1[:], p1[:], sc_sb[0:1, 0:1])
            p2 = rp.tile([1, CHUNK], _bf, tag="p2", name=f"p2_{cc}")
            eng.tensor_tensor(out=p2[:], in0=p1[:], in1=v2, op=OP.mult)
            if not zc:
                eng.tensor_scalar_add(p2[:], p2[:], sc_sb[0:1, 1:2])
            oc = rp.tile([1, CHUNK], _bf, tag="oc", name=f"oc_{cc}")
            eng.tensor_tensor(out=oc[:], in0=p2[:], in1=a3, op=OP.mult)
            return oc

        def emit_out(cc, oc, psO):
            ot = otp.tile([1, CHUNK], _f32, tag="ot", name=f"ot_{cc}")
            nc.vector.tensor_tensor(out=ot[:], in0=oc[:], in1=psO[:], op=OP.add)
            # final chunk's DMA on the (by then idle) ACT queue: its seq can
            # park on ot's semaphore while SP would still be mid-pipeline
            q = nc.scalar if cc == NCHUNK - 1 else nc.sync
            q.dma_start(out=out_d[cc:cc + 1, :], in_=ot[:])

        # x tiles / a-pass shuffles, pipelined one chunk ahead of use
        xt8s = {0: xt8_0}
        xtbs = {0: xbp.tile([128, KB, CHUNK], _bf, tag="xtb", name="xtb_0")}
        as1s = {}

        def prefetch_x(cc):
            t8 = x8p.tile([128, K8, CHUNK], _f8, tag="xt8", name=f"xt8_{cc}")
            nc.sync.dma_start(t8[:], xt8_r[:, :, cc * CHUNK:(cc + 1) * CHUNK])
            tb = xbp.tile([128, KB, CHUNK], _bf, tag="xtb", name=f"xtb_{cc}")
            nc.sync.dma_start(tb[:], xtb_r[:, :, cc * CHUNK:(cc + 1) * CHUNK])
            xt8s[cc], xtbs[cc] = t8, tb

        nc.sync.dma_start(w18_sb[:], w18_r)

        for c in range(NCHUNK):
            xt8_t = xt8s[c]
            last = c == NCHUNK - 1

            if last:
                # last chunk: a-pass ran during c-1; drain the cross combine
                # on the idle Pool right away -> tail is just psO -> ot -> DMA
                oc3 = emit_cross(c, as1s[c], nc.gpsimd)

            # ---- deep L0 (fp8 DoubleRow), psum = h0 * 4096 ----
            y0t = yp.tile([128, K8, CHUNK], _f8, tag="y0", name=f"y0_{c}")
            for m in range(M0):
                ps = dps.tile([128, CHUNK], _f32, tag="dps", name=f"ps0_{c}_{m}")
                lhs = w0l(m)
                for j in range(K8 // 2):
                    nc.tensor.matmul(
                        ps[:],
                        lhsT=lhs(j),
                        rhs=xt8_t[:, 2 * j:2 * j + 2, :],
                        start=(j == 0),
                        stop=(j == K8 // 2 - 1),
                        perf_mode=DR,
                    )
                # y0 = fp8(relu(h0)*256) = relu(psum/16 [+ 256*b0])
                relu(y0t[:, m, :], ps, 1.0 / WS,
                     None if zb else b0_sb[:, m:m + 1], on_dve=(m % 2 == 1),
                     split=(m == M0 - 1
                            and CFG["y0split"] in (1, 3 if last else 1)))

            # ---- deep L1 (fp8 DoubleRow); y1 in two pair-tiles so L2's
            # first DR matmul only waits on the first pair's relus ----
            y1p = [
                yp.tile([128, 2, CHUNK], _f8, tag=f"y1p{i}", name=f"y1p{i}_{c}")
                for i in range(M1 // 2)
            ]
            for m in range(M1):
                ps = dps.tile([128, CHUNK], _f32, tag="dps", name=f"ps1_{c}_{m}")
                for j in range(K8 // 2):
                    nc.tensor.matmul(
                        ps[:],
                        lhsT=w18_sb[:, 2 * j:2 * j + 2, m * 128:(m + 1) * 128],
                        rhs=y0t[:, 2 * j:2 * j + 2, :],
                        start=(j == 0),
                        stop=(j == K8 // 2 - 1),
                        perf_mode=DR,
                    )
                relu(
                    y1p[m // 2][:, m % 2, :], ps, 1.0 / WS,
                    None if zb else b1_sb[:, m:m + 1], on_dve=(m % 2 == 1),
                    split=(m == M1 - 1
                           and CFG["l1split"] in (1, 3 if last else 1)),
                )

            # ---- the L1->L2 bubble (PE waits on y1 relus): prefetch DMAs
            # and a-passes fill it.  Order keeps dependency-gated small DMAs
            # (as1) from blocking bulk transfers that are needed sooner. ----
            if c == 0:
                nc.sync.dma_start(xtbs[0][:], xtb_r[:, :, 0:CHUNK])
                t8 = x8p.tile([128, K8, CHUNK], _f8, tag="xt8", name="xt8_1")
                nc.sync.dma_start(t8[:], xt8_r[:, :, CHUNK:2 * CHUNK])
                xt8s[1] = t8
                nc.sync.dma_start(w28_sb[:], w28_r)
                as1s[0] = emit_apass(0, xtbs[0])
                tb = xbp.tile([128, KB, CHUNK], _bf, tag="xtb", name="xtb_1")
                nc.sync.dma_start(tb[:], xtb_r[:, :, CHUNK:2 * CHUNK])
                xtbs[1] = tb
            elif c == 1:
                as1s[1] = emit_apass(1, xtbs[1])
                prefetch_x(2)
                prefetch_x(3)
            elif c == 2:
                as1s[2] = emit_apass(2, xtbs[2])
                as1s[3] = emit_apass(3, xtbs[3])

            # ---- deep L2 (fp8 DoubleRow) -> bf16 y2 (natural scale) ----
            y2t = yp.tile([128, M2, CHUNK], _bf, tag="y2", name=f"y2_{c}")
            for m in range(M2):
                ps = dps.tile([128, CHUNK], _f32, tag="dps", name=f"ps2_{c}_{m}")
                for j in range(M1 // 2):
                    nc.tensor.matmul(
                        ps[:],
                        lhsT=w28_sb[:, 2 * j:2 * j + 2, m * 128:(m + 1) * 128],
                        rhs=y1p[j][:, :, :],
                        start=(j == 0),
                        stop=(j == M1 // 2 - 1),
                        perf_mode=DR,
                    )
                relu(
                    y2t[:, m, :], ps, 1.0 / (XS * WS),
                    None if zb else b2_sb[:, m:m + 1],
                    on_dve=(m % 2 == 1 if CFG["l2_dve"] == "alt" else False),
                    split=(CFG["l2split"] in (1, 3 if last else 1)),
                )

            # ---- out_d: psO = y_deep . ow_d (+ obP via ones-matmul) ----
            psO = ops.tile([1, CHUNK], _f32, tag="po", name=f"psO_{c}")
            for m in range(M2):
                nc.tensor.matmul(
                    psO[:],
                    lhsT=owd(m),
                    rhs=y2t[:, m, :],
                    start=(m == 0),
                    stop=(m == M2 - 1) and zo,
                )
            if not zo:
                nc.tensor.matmul(
                    psO[:], lhsT=obb, rhs=ones_sb[:], start=False, stop=True
                )

            # ---- chunk epilogue: cross combine + output ----
            if last:
                emit_out(c, oc3, psO)
            else:
                oc = emit_cross(c, as1s[c], nc.vector)
                emit_out(c, oc, psO)

    nc.compile()
    return nc


def _get_nc(zb=True, zc=True, zo=True):
    key = f"nc_zb{int(zb)}_zc{int(zc)}_zo{int(zo)}"
    if key not in _CACHE:
        _CACHE[key] = _build_nc(zb=zb, zc=zc, zo=zo)
    return _CACHE[key]


def _prep_in_maps(inputs, zb, zc, zo):
    fi = np.asarray(inputs["feature_index"]).astype(np.int64)
    fvv = np.asarray(inputs["feature_value"], dtype=np.float32)
    with_fv = not bool(np.all(fvv == 1.0))
    emb = np.asarray(inputs["emb_table"], dtype=np.float32)
    cw = np.asarray(inputs["cross_w"], dtype=np.float32)
    cb = np.asarray(inputs["cross_b"], dtype=np.float32)
    w0 = np.asarray(inputs["w0"], dtype=np.float32)
    b0 = np.asarray(inputs["b0"], dtype=np.float32)
    w1 = np.asarray(inputs["w1"], dtype=np.float32)
    b1 = np.asarray(inputs["b1"], dtype=np.float32)
    w2 = np.asarray(inputs["w2"], dtype=np.float32)
    b2 = np.asarray(inputs["b2"], dtype=np.float32)
    ow = np.asarray(inputs["out_w"], dtype=np.float32).reshape(-1)
    ob = np.asarray(inputs["out_b"], dtype=np.float32).reshape(-1)

    # ---- host gather into padded, transposed layouts ----
    idxb = np.full((B, FPB), NF, dtype=np.int64)
    idxb[:, :F] = fi
    idx8 = np.full((B, FP8), NF, dtype=np.int64)
    idx8[:, :F] = fi
    if with_fv:
        embp = np.zeros((NF + 1, E), dtype=np.float32)
        embp[:NF] = emb
        xb_nat = embp[idxb]                       # [B, 28, 32] f32
        xb_nat *= np.concatenate(
            [fvv, np.ones((B, FPB - F), np.float32)], axis=1
        )[:, :, None]
        x8_nat = np.zeros((B, FP8, E), dtype=np.float32)
        x8_nat[:, :FPB] = xb_nat
        x8_nat = (x8_nat * XS).astype(_np_f8)
        xb_nat = xb_nat.astype(_np_bf)
    else:
        table_bf = np.zeros((NF + 1, E), dtype=_np_bf)
        table_bf[:NF] = emb.astype(_np_bf)
        table_f8 = np.zeros((NF + 1, E), dtype=_np_f8)
        table_f8[:NF] = (emb * XS).astype(_np_f8)
        xb_nat = table_bf[idxb]                   # [B, 28, 32] bf16
        x8_nat = table_f8[idx8]                   # [B, 32, 32] fp8

    # ---- shared (replicated) weight layouts ----
    def kpm(w, ktiles, scale):
        # [K, M] -> [128, ktiles*M] with w[p, k, m] = W[k*128+p, m]*scale
        K, M = w.shape
        wq = np.zeros((ktiles * 128, M), dtype=np.float32)
        wq[:K] = w * scale
        return np.ascontiguousarray(
            wq.reshape(ktiles, 128, M).transpose(1, 0, 2).reshape(128, ktiles * M)
        )

    w08 = kpm(w0, K8, WS).astype(_np_f8)
    w18 = kpm(w1, K8, WS).astype(_np_f8)
    w28 = kpm(w2, M1, WS).astype(_np_f8)

    wl = np.zeros((4, KB * 128), dtype=np.float32)
    wl[0, :D] = cw[0]
    wl[1, :D] = cw[1]
    wl[2, :D] = cw[2]
    wl[3, :D] = ow[:D]
    cwo = wl.reshape(4, KB, 128).transpose(2, 1, 0).reshape(128, KB * 4)
    owd = ow[D:].reshape(M2, 128).T
    C = np.cumsum(cb)
    obb = np.zeros((128, 1), dtype=np.float32)
    obb[0, 0] = ob[0] + C[2] * ow[:D].sum()
    vbc = np.zeros((128, 1), dtype=np.float32)
    vbc[0:3, 0] = 1.0
    smw = np.ascontiguousarray(
        np.concatenate([cwo, owd, obb, vbc], axis=1)
    ).astype(_np_bf)

    shared = dict(w08=w08, w18=w18, w28=w28, smw=smw)
    if not zc:
        shared["sc"] = np.array(
            [[C[0] * cw[1].sum(), C[1] * cw[2].sum()]], dtype=np.float32
        )
    if not zb:
        b0r = (b0 * XS).reshape(M0, 128).T
        b1r = (b1 * XS).reshape(M1, 128).T
        b2r = (b2 * XS).reshape(M2, 128).T
        shared["cst"] = np.ascontiguousarray(
            np.concatenate([b0r, b1r, b2r], axis=1).astype(np.float32)
        )

    in_maps = []
    for core in range(N_CORES):
        sl = slice(core * S, (core + 1) * S)
        # [S, K, 128] -> [128, K, S] -> [128, K*S]
        xtb = np.ascontiguousarray(
            xb_nat[sl].reshape(S, KB, 128).transpose(2, 1, 0).reshape(128, KB * S)
        )
        xt8 = np.ascontiguousarray(
            x8_nat[sl].reshape(S, K8, 128).transpose(2, 1, 0).reshape(128, K8 * S)
        )
        in_maps.append(dict(xtb=xtb, xt8=xt8, **shared))
    return in_maps


def _flags(inputs):
    zb = (
        bool(np.all(np.asarray(inputs["b0"]) == 0.0))
        and bool(np.all(np.asarray(inputs["b1"]) == 0.0))
        and bool(np.all(np.asarray(inputs["b2"]) == 0.0))
    )
    zc = bool(np.all(np.asarray(inputs["cross_b"]) == 0.0))
    ow = np.asarray(inputs["out_w"], dtype=np.float32).reshape(-1)
    cb = np.asarray(inputs["cross_b"], dtype=np.float32)
    obp = float(np.asarray(inputs["out_b"]).reshape(-1)[0]) + float(
        np.cumsum(cb)[2] * ow[:D].sum()
    )
    zo = obp == 0.0
    return zb, zc, zo


def _run(inputs, trace=False, **kw):
    zb, zc, zo = _flags(inputs)
    nc = _get_nc(zb=zb, zc=zc, zo=zo)
    in_maps = _prep_in_maps(inputs, zb, zc, zo)
    res = run_bass_kernel_spmd(
        nc, in_maps, core_ids=list(range(N_CORES)), trace=trace, **kw
    )
    out = np.concatenate([r["out"].reshape(S, 1) for r in res.results], axis=0)
    return out.astype(np.float32), res


def kernel(**inputs) -> np.ndarray:
    out, _ = _run(inputs, trace=False)
    return out


# revision 94
# speedup vs baseline: 1.0138x; 1.0138x over previous
"""DCN (cross+deep) Trainium2 Bass kernel, 8 NeuronCores.

Sharding: data-parallel over batch (2048 rows/core); embedding gather on
host (table never touches the device); cross/deep weights replicated.

Key structure (vs the naive formulation):
  * Cross branch is algebraically collapsed: with a_i = x0 . w_i and
    a_3 = x0 . ow_cross, the full cross stack + its output contribution
    reduce to per-row scalar recurrences:
       S0 = a0; u1 = 1+S0; S1 = u1*a1 + c1; u2 = u1+S1; S2 = u2*a2 + c2;
       T = u2+S2; out_cross = T*a3 + const.
    So the PE computes ONE 7-matmul group ([128,4] lhsT) instead of
    3x7 broadcast matvecs + 7 output matvecs.
  * Deep branch runs in fp8(e4m3) with DoubleRow perf mode: each matmul
    contracts two 128-row k-tiles at 0.5 cycles/output-row. Activations
    are scaled x256 and weights x16 (exact power-of-2 descale in the
    relus), keeping everything in e4m3's normal range.
  * x ships pre-transposed from host in bf16 (cross) + fp8 (deep)
    layouts. Engine split per chunk: ACT = L0 relus + a-copy; DVE =
    L1/L2 relus + final add; Pool = cross recurrence ([4,128] layout,
    brought to partitions 0-3 by a tiny SBUF->SBUF DMA shuffle).
  * L1/L2 run k-pair-outer so they can start as soon as the first two
    producer tiles are relu'd; out_d accumulates in [4,128] PSUM groups
    so the tail is one small DVE add + DMA.
"""

import numpy as np
import ml_dtypes
from contextlib import ExitStack

import concourse.tile as tile
import concourse.mybir as mybir
from concourse import bacc
from concourse.bass_utils import run_bass_kernel_spmd

# ---- problem constants (hardcoded; kernel.py must be self-contained) ----
B, F, E = 16384, 26, 32
NF = 1_000_000
D = F * E                     # 832
DEEP = (1024, 512, 256)
N_CORES = 8
S = B // N_CORES              # 2048 rows per core
CHUNK = 512
NCHUNK = S // CHUNK           # 4
KB = 7                        # bf16 k-tiles (896 = 28 features)
K8 = 8                        # fp8 k-tiles (1024 = 32 features)
FPB, FP8 = 28, 32             # padded feature counts
M0, M1, M2 = DEEP[0] // 128, DEEP[1] // 128, DEEP[2] // 128  # 8, 4, 2
XS, WS = 256.0, 16.0          # fp8 scales for activations / weights

_bf = mybir.dt.bfloat16
_f8 = mybir.dt.float8e4
_f32 = mybir.dt.float32
_np_bf = ml_dtypes.bfloat16
_np_f8 = ml_dtypes.float8_e4m3

_CACHE = {}
DR = mybir.MatmulPerfMode.DoubleRow


# scheduling knobs (swept against the cost-model timeline sim)
# *split: 0 = never, 1 = all chunks, 3 = last chunk only
CFG = dict(l2_dve="alt", y0split=0, l1split=0, l2split=0, dpsb=5, warm_n=7)


def _build_nc(zb=True, zc=True, zo=True):
    """zb: deep biases all zero; zc: cross biases zero; zo: out bias zero."""
    AF = mybir.ActivationFunctionType
    OP = mybir.AluOpType
    nc = bacc.Bacc(
        "TRN2", target_bir_lowering=False, debug=False, num_devices=N_CORES
    )

    # x pre-transposed on host: xtb[p, k*S+b] = bf16(x[b, k*128+p])
    xtb_d = nc.dram_tensor("xtb", [128, KB * S], _bf, kind="ExternalInput")
    # x8[p, k*S+b] = fp8(x[b, k*128+p] * 256)
    xt8_d = nc.dram_tensor("xt8", [128, K8 * S], _f8, kind="ExternalInput")
    # deep weights fp8 (x16): w[p, k, m] = fp8(W[k*128+p, m] * 16)
    w08_d = nc.dram_tensor("w08", [128, K8 * DEEP[0]], _f8, kind="ExternalInput")
    w18_d = nc.dram_tensor("w18", [128, K8 * DEEP[1]], _f8, kind="ExternalInput")
    w28_d = nc.dram_tensor("w28", [128, M1 * DEEP[2]], _f8, kind="ExternalInput")

    # merged small weights bf16: [cwo (28) | owd (2) | obb (1) | vb (1)]
    # vb: rows 0-2 = 1.0, row 3 = 0 -- the asb-copy's per-partition bias,
    # so asb rows become [v0,v1,v2,a3] with v_i = 1 + a_i and the cross
    # recurrence factorizes to out_cross = ((v0*v1 + c1)*v2 + c2) * a3.
    SMW = KB * 4 + M2 + 1 + 1
    smw_d = nc.dram_tensor("smw", [128, SMW], _bf, kind="ExternalInput")
    if not zc:
        sc_d = nc.dram_tensor("sc", [1, 2], _f32, kind="ExternalInput")
    if not zb:
        cst_d = nc.dram_tensor("cst", [128, M0 + M1 + M2], _f32, kind="ExternalInput")
    out_d = nc.dram_tensor("out", [NCHUNK, CHUNK], _f32, kind="ExternalOutput")

    xtb_r = xtb_d[:, :].rearrange("p (k s) -> p k s", k=KB)
    xt8_r = xt8_d[:, :].rearrange("p (k s) -> p k s", k=K8)
    w08_r = w08_d[:, :].rearrange("p (k m) -> p k m", k=K8)
    w18_r = w18_d[:, :].rearrange("p (k m) -> p k m", k=K8)
    w28_r = w28_d[:, :].rearrange("p (k m) -> p k m", k=M1)

    with ExitStack() as ctx:
        tc = ctx.enter_context(tile.TileContext(nc))
        wp = ctx.enter_context(tc.tile_pool(name="wp", bufs=1))
        xbp = ctx.enter_context(tc.tile_pool(name="xbp", bufs=2))
        x8p = ctx.enter_context(tc.tile_pool(name="x8p", bufs=2))
        yp = ctx.enter_context(tc.tile_pool(name="yp", bufs=2))
        asp = ctx.enter_context(tc.tile_pool(name="asp", bufs=2))
        rp = ctx.enter_context(tc.tile_pool(name="rp", bufs=2))
        otp = ctx.enter_context(tc.tile_pool(name="otp", bufs=2))
        dps = ctx.enter_context(
            tc.tile_pool(name="dps", bufs=CFG["dpsb"], space="PSUM")
        )
        aps = ctx.enter_context(tc.tile_pool(name="aps", bufs=1, space="PSUM"))
        ops = ctx.enter_context(tc.tile_pool(name="ops", bufs=1, space="PSUM"))

        # ---- startup DMA order: smw (tiny, touched by PE preamble), then
        # x chunk 0 + w0 (the L0 critical path).  w0 lives in TWO tiles so
        # L0 m0-3 don't wait on the second DMA (tile dependency tracking is
        # tile-granular). ----
        w08a_sb = wp.tile([128, K8, DEEP[0] // 2], _f8)
        w08b_sb = wp.tile([128, K8, DEEP[0] // 2], _f8)
        w18_sb = wp.tile([128, K8, DEEP[1]], _f8)
        w28_sb = wp.tile([128, M1, DEEP[2]], _f8)
        nc.sync.dma_start(w08a_sb[:], w08_r[:, :, 0:512])
        xt8_0 = x8p.tile([128, K8, CHUNK], _f8, tag="xt8", name="xt8_0")
        nc.sync.dma_start(xt8_0[:], xt8_r[:, :, 0:CHUNK])
        nc.sync.dma_start(w08b_sb[:], w08_r[:, :, 512:1024])
        smw_sb = wp.tile([128, SMW], _bf)
        nc.sync.dma_start(smw_sb[:], smw_d[:, :])

        def w0l(m):  # [128, 2, 128] lhsT slice provider for L0 tile (j pair)
            t = w08a_sb if m < 4 else w08b_sb
            mm = m % 4
            return lambda j: t[:, 2 * j:2 * j + 2, mm * 128:(mm + 1) * 128]

        def cwo(k):  # [128, 4] lhsT for a-pass k-tile
            return smw_sb[:, k * 4:(k + 1) * 4]

        def owd(m):  # [128, 1] deep-out column
            return smw_sb[:, KB * 4 + m:KB * 4 + m + 1]

        obb = smw_sb[:, KB * 4 + M2:KB * 4 + M2 + 1]
        vb = smw_sb[0:4, KB * 4 + M2 + 1:KB * 4 + M2 + 2]
        if not zc:
            sc_sb = wp.tile([1, 2], _f32)
            nc.sync.dma_start(sc_sb[:], sc_d[:, :])
        if not zb:
            cst_sb = wp.tile([128, M0 + M1 + M2], _f32)
            nc.sync.dma_start(cst_sb[:], cst_d[:, :])
            b0_sb = cst_sb[:, 0:M0]
            b1_sb = cst_sb[:, M0:M0 + M1]
            b2_sb = cst_sb[:, M0 + M1:M0 + M1 + M2]

        # ---- preamble: observe ops + PE warm-up (p-state ramp) ----
        obs = wp.tile([128, 8], _f32)
        nc.vector.tensor_copy(obs[:, 0:1], smw_sb[:, 0:1])
        nc.gpsimd.tensor_copy(obs[:, 1:2], smw_sb[:, 0:1])
        if not zc:
            nc.vector.tensor_copy(obs[:, 2:3], sc_sb[0:1, 0:1])
        nc.scalar.activation(obs[:, 3:4], smw_sb[:, 0:1], AF.Copy)
        if not zb:
            nc.scalar.activation(obs[:, 4:5], b0_sb[:, 0:1], AF.Copy)
        warm = wp.tile([128, 512], _bf)
        nc.vector.memset(warm[:], 0.0)
        if not zo:
            ones_sb = wp.tile([128, CHUNK], _bf)
            nc.gpsimd.memset(ones_sb[:], 1.0)
        warm_ps = dps.tile([128, 512], _f32, tag="dps", name="warm_ps")
        for _ in range(CFG["warm_n"]):
            nc.tensor.matmul(
                warm_ps[:], lhsT=warm[:, 0:128], rhs=warm[:], start=True, stop=True
            )
        # NOTE: only touch tensors whose DMAs are emitted BEFORE this point —
        # touching late-loaded weights stalls the in-order PE stream on their
        # DMA semaphores.
        dummy_ps = ops.tile([1, 8], _f32, tag="dummy", bufs=1)
        touch = [
            w08a_sb[:, 0:1, 0:1],
        ]
        if not zo:
            touch.append(ones_sb[:, 0:1])
        for w_ap in touch:
            nc.tensor.matmul(dummy_ps[0:1, 0:1], lhsT=w_ap, rhs=w_ap, start=True, stop=True)

        HH = CHUNK // 2

        def relu(out_ap, ps, scale, bias_col, on_dve, split=False):
            # relu of one [128, CHUNK] psum tile; tiles alternate between ACT
            # and DVE so neither engine queues (inter-arrival 2x427ns > op).
            # split=True: column-halves on both engines -> ~2x lower latency,
            # for tiles that gate the next layer's last matmuls.
            if zb and split:
                nc.scalar.activation(
                    out_ap[:, 0:HH], ps[:, 0:HH], AF.Relu, scale=scale
                )
                nc.vector.tensor_scalar(
                    out_ap[:, HH:], ps[:, HH:], scale, 0.0, OP.mult, OP.max
                )
            elif zb and on_dve:
                nc.vector.tensor_scalar(
                    out_ap[:, :], ps[:, :], scale, 0.0, OP.mult, OP.max
                )
            else:
                nc.scalar.activation(
                    out_ap[:, :], ps[:, :], AF.Relu,
                    bias=0.0 if bias_col is None else bias_col, scale=scale,
                )

        def emit_apass(cc, xtb_t):
            # cross a-pass (bf16): psA rows = [a0, a1, a2, a3]; the +1
            # for v_i = 1 + a_i rides the asb copy as a partition bias
            psA = aps.tile([4, CHUNK], _f32, tag="a", name=f"psA_{cc}")
            for k in range(KB):
                nc.tensor.matmul(
                    psA[:],
                    lhsT=cwo(k),
                    rhs=xtb_t[:, k, :],
                    start=(k == 0),
                    stop=(k == KB - 1),
                )
            asb = asp.tile([4, CHUNK], _bf, tag="asb", name=f"asb_{cc}")
            nc.scalar.activation(asb[:], psA[:], AF.Identity, bias=vb)
            # shuffle all four rows onto partition 0 (engines can't cross
            # partitions; the DMA crossbar can): as1[0,i,b] = a_i[b]
            as1 = asp.tile([1, 4, CHUNK], _bf, tag="as1", name=f"as1_{cc}")
            nc.sync.dma_start(out=as1[:, :, :], in_=asb[:, :])
            return as1

        def emit_cross(cc, as1, eng):
            # cross combine: oc = ((v0*v1 + c1)*v2 + c2) * a3.  DVE at chunk
            # end for pipelined chunks; Pool (idle) for the last chunk so its
            # relus aren't queued behind the chain.
            v0 = as1[:, 0, :]
            v1 = as1[:, 1, :]
            v2 = as1[:, 2, :]
            a3 = as1[:, 3, :]
            p1 = rp.tile([1, CHUNK], _bf, tag="p1", name=f"p1_{cc}")
            eng.tensor_tensor(out=p1[:], in0=v0, in1=v1, op=OP.mult)
            if not zc:
                eng.tensor_scalar_add(p1[:], p1[:], sc_sb[0:1, 0:1])
            p2 = rp.tile([1, CHUNK], _bf, tag="p2", name=f"p2_{cc}")
            eng.tensor_tensor(out=p2[:], in0=p1[:], in1=v2, op=OP.mult)
            if not zc:
                eng.tensor_scalar_add(p2[:], p2[:], sc_sb[0:1, 1:2])
            oc = rp.tile([1, CHUNK], _bf, tag="oc", name=f"oc_{cc}")
            eng.tensor_tensor(out=oc[:], in0=p2[:], in1=a3, op=OP.mult)
            return oc

        def emit_out(cc, oc, psO):
            ot = otp.tile([1, CHUNK], _f32, tag="ot", name=f"ot_{cc}")
            nc.vector.tensor_tensor(out=ot[:], in0=oc[:], in1=psO[:], op=OP.add)
            # final chunk's DMA on the (by then idle) ACT queue: its seq can
            # park on ot's semaphore while SP would still be mid-pipeline
            q = nc.scalar if cc == NCHUNK - 1 else nc.sync
            q.dma_start(out=out_d[cc:cc + 1, :], in_=ot[:])

        # x tiles / a-pass shuffles, pipelined one chunk ahead of use
        xt8s = {0: xt8_0}
        xtbs = {0: xbp.tile([128, KB, CHUNK], _bf, tag="xtb", name="xtb_0")}
        as1s = {}

        def prefetch_x(cc):
            t8 = x8p.tile([128, K8, CHUNK], _f8, tag="xt8", name=f"xt8_{cc}")
            nc.sync.dma_start(t8[:], xt8_r[:, :, cc * CHUNK:(cc + 1) * CHUNK])
            tb = xbp.tile([128, KB, CHUNK], _bf, tag="xtb", name=f"xtb_{cc}")
            nc.sync.dma_start(tb[:], xtb_r[:, :, cc * CHUNK:(cc + 1) * CHUNK])
            xt8s[cc], xtbs[cc] = t8, tb

        nc.sync.dma_start(w18_sb[:], w18_r)

        for c in range(NCHUNK):
            xt8_t = xt8s[c]
            last = c == NCHUNK - 1

            if last:
                # last chunk: a-pass ran during c-1; drain the cross combine
                # on the idle Pool right away -> tail is just psO -> ot -> DMA
                oc3 = emit_cross(c, as1s[c], nc.gpsimd)

            # ---- deep L0 (fp8 DoubleRow), psum = h0 * 4096 ----
            y0t = yp.tile([128, K8, CHUNK], _f8, tag="y0", name=f"y0_{c}")
            for m in range(M0):
                ps = dps.tile([128, CHUNK], _f32, tag="dps", name=f"ps0_{c}_{m}")
                lhs = w0l(m)
                for j in range(K8 // 2):
                    nc.tensor.matmul(
                        ps[:],
                        lhsT=lhs(j),
                        rhs=xt8_t[:, 2 * j:2 * j + 2, :],
                        start=(j == 0),
                        stop=(j == K8 // 2 - 1),
                        perf_mode=DR,
                    )
                # y0 = fp8(relu(h0)*256) = relu(psum/16 [+ 256*b0])
                relu(y0t[:, m, :], ps, 1.0 / WS,
                     None if zb else b0_sb[:, m:m + 1], on_dve=(m % 2 == 1),
                     split=(m == M0 - 1
                            and CFG["y0split"] in (1, 3 if last else 1)))

            # ---- deep L1 (fp8 DoubleRow); y1 in two pair-tiles so L2's
            # first DR matmul only waits on the first pair's relus ----
            y1p = [
                yp.tile([128, 2, CHUNK], _f8, tag=f"y1p{i}", name=f"y1p{i}_{c}")
                for i in range(M1 // 2)
            ]
            for m in range(M1):
                ps = dps.tile([128, CHUNK], _f32, tag="dps", name=f"ps1_{c}_{m}")
                for j in range(K8 // 2):
                    nc.tensor.matmul(
                        ps[:],
                        lhsT=w18_sb[:, 2 * j:2 * j + 2, m * 128:(m + 1) * 128],
                        rhs=y0t[:, 2 * j:2 * j + 2, :],
                        start=(j == 0),
                        stop=(j == K8 // 2 - 1),
                        perf_mode=DR,
                    )
                relu(
                    y1p[m // 2][:, m % 2, :], ps, 1.0 / WS,
                    None if zb else b1_sb[:, m:m + 1], on_dve=(m % 2 == 1),
                    split=(m == M1 - 1
                           and CFG["l1split"] in (1, 3 if last else 1)),
                )

            # ---- the L1->L2 bubble (PE waits on y1 relus): prefetch DMAs
            # and a-passes fill it.  Order keeps dependency-gated small DMAs
            # (as1) from blocking bulk transfers that are needed sooner. ----
            if c == 0:
                nc.sync.dma_start(xtbs[0][:], xtb_r[:, :, 0:CHUNK])
                t8 = x8p.tile([128, K8, CHUNK], _f8, tag="xt8", name="xt8_1")
                nc.sync.dma_start(t8[:], xt8_r[:, :, CHUNK:2 * CHUNK])
                xt8s[1] = t8
                nc.sync.dma_start(w28_sb[:], w28_r)
                as1s[0] = emit_apass(0, xtbs[0])
                tb = xbp.tile([128, KB, CHUNK], _bf, tag="xtb", name="xtb_1")
                nc.sync.dma_start(tb[:], xtb_r[:, :, CHUNK:2 * CHUNK])
                xtbs[1] = tb
            elif c == 1:
                as1s[1] = emit_apass(1, xtbs[1])
                prefetch_x(2)
                prefetch_x(3)
            elif c == 2:
                as1s[2] = emit_apass(2, xtbs[2])
                as1s[3] = emit_apass(3, xtbs[3])

            # ---- deep L2 (fp8 DoubleRow) -> bf16 y2 (natural scale) ----
            y2t = yp.tile([128, M2, CHUNK], _bf, tag="y2", name=f"y2_{c}")
            for m in range(M2):
                ps = dps.tile([128, CHUNK], _f32, tag="dps", name=f"ps2_{c}_{m}")
                for j in range(M1 // 2):
                    nc.tensor.matmul(
                        ps[:],
                        lhsT=w28_sb[:, 2 * j:2 * j + 2, m * 128:(m + 1) * 128],
                        rhs=y1p[j][:, :, :],
                        start=(j == 0),
                        stop=(j == M1 // 2 - 1),
                        perf_mode=DR,
                    )
                relu(
                    y2t[:, m, :], ps, 1.0 / (XS * WS),
                    None if zb else b2_sb[:, m:m + 1],
                    on_dve=(m % 2 == 1 if CFG["l2_dve"] == "alt" else False),
                    split=(CFG["l2split"] in (1, 3 if last else 1)),
                )

            # ---- out_d: psO = y_deep . ow_d (+ obP via ones-matmul) ----
            psO = ops.tile([1, CHUNK], _f32, tag="po", name=f"psO_{c}")
            for m in range(M2):
                nc.tensor.matmul(
                    psO[:],
                    lhsT=owd(m),
                    rhs=y2t[:, m, :],
                    start=(m == 0),
                    stop=(m == M2 - 1) and zo,
                )
            if not zo:
                nc.tensor.matmul(
                    psO[:], lhsT=obb, rhs=ones_sb[:], start=False, stop=True
                )

            # ---- chunk epilogue: cross combine + output ----
            if last:
                emit_out(c, oc3, psO)
            else:
                oc = emit_cross(c, as1s[c], nc.vector)
                emit_out(c, oc, psO)

    nc.compile()
    return nc


def _get_nc(zb=True, zc=True, zo=True):
    key = f"nc_zb{int(zb)}_zc{int(zc)}_zo{int(zo)}"
    if key not in _CACHE:
        _CACHE[key] = _build_nc(zb=zb, zc=zc, zo=zo)
    return _CACHE[key]


def _prep_in_maps(inputs, zb, zc, zo):
    fi = np.asarray(inputs["feature_index"]).astype(np.int64)
    fvv = np.asarray(inputs["feature_value"], dtype=np.float32)
    with_fv = not bool(np.all(fvv == 1.0))
    emb = np.asarray(inputs["emb_table"], dtype=np.float32)
    cw = np.asarray(inputs["cross_w"], dtype=np.float32)
    cb = np.asarray(inputs["cross_b"], dtype=np.float32)
    w0 = np.asarray(inputs["w0"], dtype=np.float32)
    b0 = np.asarray(inputs["b0"], dtype=np.float32)
    w1 = np.asarray(inputs["w1"], dtype=np.float32)
    b1 = np.asarray(inputs["b1"], dtype=np.float32)
    w2 = np.asarray(inputs["w2"], dtype=np.float32)
    b2 = np.asarray(inputs["b2"], dtype=np.float32)
    ow = np.asarray(inputs["out_w"], dtype=np.float32).reshape(-1)
    ob = np.asarray(inputs["out_b"], dtype=np.float32).reshape(-1)

    # ---- host gather into padded, transposed layouts ----
    idxb = np.full((B, FPB), NF, dtype=np.int64)
    idxb[:, :F] = fi
    idx8 = np.full((B, FP8), NF, dtype=np.int64)
    idx8[:, :F] = fi
    if with_fv:
        embp = np.zeros((NF + 1, E), dtype=np.float32)
        embp[:NF] = emb
        xb_nat = embp[idxb]                       # [B, 28, 32] f32
        xb_nat *= np.concatenate(
            [fvv, np.ones((B, FPB - F), np.float32)], axis=1
        )[:, :, None]
        x8_nat = np.zeros((B, FP8, E), dtype=np.float32)
        x8_nat[:, :FPB] = xb_nat
        x8_nat = (x8_nat * XS).astype(_np_f8)
        xb_nat = xb_nat.astype(_np_bf)
    else:
        table_bf = np.zeros((NF + 1, E), dtype=_np_bf)
        table_bf[:NF] = emb.astype(_np_bf)
        table_f8 = np.zeros((NF + 1, E), dtype=_np_f8)
        table_f8[:NF] = (emb * XS).astype(_np_f8)
        xb_nat = table_bf[idxb]                   # [B, 28, 32] bf16
        x8_nat = table_f8[idx8]                   # [B, 32, 32] fp8

    # ---- shared (replicated) weight layouts ----
    def kpm(w, ktiles, scale):
        # [K, M] -> [128, ktiles*M] with w[p, k, m] = W[k*128+p, m]*scale
        K, M = w.shape
        wq = np.zeros((ktiles * 128, M), dtype=np.float32)
        wq[:K] = w * scale
        return np.ascontiguousarray(
            wq.reshape(ktiles, 128, M).transpose(1, 0, 2).reshape(128, ktiles * M)
        )

    w08 = kpm(w0, K8, WS).astype(_np_f8)
    w18 = kpm(w1, K8, WS).astype(_np_f8)
    w28 = kpm(w2, M1, WS).astype(_np_f8)

    wl = np.zeros((4, KB * 128), dtype=np.float32)
    wl[0, :D] = cw[0]
    wl[1, :D] = cw[1]
    wl[2, :D] = cw[2]
    wl[3, :D] = ow[:D]
    cwo = wl.reshape(4, KB, 128).transpose(2, 1, 0).reshape(128, KB * 4)
    owd = ow[D:].reshape(M2, 128).T
    C = np.cumsum(cb)
    obb = np.zeros((128, 1), dtype=np.float32)
    obb[0, 0] = ob[0] + C[2] * ow[:D].sum()
    vbc = np.zeros((128, 1), dtype=np.float32)
    vbc[0:3, 0] = 1.0
    smw = np.ascontiguousarray(
        np.concatenate([cwo, owd, obb, vbc], axis=1)
    ).astype(_np_bf)

    shared = dict(w08=w08, w18=w18, w28=w28, smw=smw)
    if not zc:
        shared["sc"] = np.array(
            [[C[0] * cw[1].sum(), C[1] * cw[2].sum()]], dtype=np.float32
        )
    if not zb:
        b0r = (b0 * XS).reshape(M0, 128).T
        b1r = (b1 * XS).reshape(M1, 128).T
        b2r = (b2 * XS).reshape(M2, 128).T
        shared["cst"] = np.ascontiguousarray(
            np.concatenate([b0r, b1r, b2r], axis=1).astype(np.float32)
        )

    in_maps = []
    for core in range(N_CORES):
        sl = slice(core * S, (core + 1) * S)
        # [S, K, 128] -> [128, K, S] -> [128, K*S]
        xtb = np.ascontiguousarray(
            xb_nat[sl].reshape(S, KB, 128).transpose(2, 1, 0).reshape(128, KB * S)
        )
        xt8 = np.ascontiguousarray(
            x8_nat[sl].reshape(S, K8, 128).transpose(2, 1, 0).reshape(128, K8 * S)
        )
        in_maps.append(dict(xtb=xtb, xt8=xt8, **shared))
    return in_maps


def _flags(inputs):
    zb = (
        bool(np.all(np.asarray(inputs["b0"]) == 0.0))
        and bool(np.all(np.asarray(inputs["b1"]) == 0.0))
        and bool(np.all(np.asarray(inputs["b2"]) == 0.0))
    )
    zc = bool(np.all(np.asarray(inputs["cross_b"]) == 0.0))
    ow = np.asarray(inputs["out_w"], dtype=np.float32).reshape(-1)
    cb = np.asarray(inputs["cross_b"], dtype=np.float32)
    obp = float(np.asarray(inputs["out_b"]).reshape(-1)[0]) + float(
        np.cumsum(cb)[2] * ow[:D].sum()
    )
    zo = obp == 0.0
    return zb, zc, zo


def _run(inputs, trace=False, **kw):
    zb, zc, zo = _flags(inputs)
    nc = _get_nc(zb=zb, zc=zc, zo=zo)
    in_maps = _prep_in_maps(inputs, zb, zc, zo)
    res = run_bass_kernel_spmd(
        nc, in_maps, core_ids=list(range(N_CORES)), trace=trace, **kw
    )
    out = np.concatenate([r["out"].reshape(S, 1) for r in res.results], axis=0)
    return out.astype(np.float32), res


def kernel(**inputs) -> np.ndarray:
    out, _ = _run(inputs, trace=False)
    return out
